# revision 1
# baseline (speedup 1.0000x reference)
import sys, os
sys.path.insert(0, '/opt/trn_rl_repo')
import numpy as np
import ml_dtypes

from contextlib import ExitStack
import concourse.bass as bass
import concourse.mybir as mybir
import concourse.tile as tile
from concourse import bacc
from concourse.bass_utils import run_bass_kernel_spmd

F32 = mybir.dt.float32
BF16 = mybir.dt.bfloat16
AF = mybir.ActivationFunctionType
OP = mybir.AluOpType

HEADS, DH, CD = 8, 64, 512
B, H, W = 4, 128, 128
NLOC, NEXT = 8192, 8704  # core cols, + 2x256 halo cols at the END
NCORES = 8
EPS = 1e-12

_cache = {}


def _emit(nc, tc):
    ctx = ExitStack()
    ident_d = nc.dram_tensor("ident", [128, 128], F32, kind="ExternalInput")
    xn_d = nc.dram_tensor("xn", [NLOC, CD], F32, kind="ExternalInput")
    xt_d = nc.dram_tensor("xt", [CD, NEXT], F32, kind="ExternalInput")
    mk_d = nc.dram_tensor("mk", [CD, NLOC], BF16, kind="ExternalInput")
    wkq_d = nc.dram_tensor("wkq", [CD, 1024], F32, kind="ExternalInput")
    wv_d = nc.dram_tensor("wv", [CD, CD], F32, kind="ExternalInput")
    wp_d = nc.dram_tensor("wp", [HEADS, DH, CD], F32, kind="ExternalInput")
    w1c_d = nc.dram_tensor("w1c", [CD, 9], F32, kind="ExternalInput")
    w2c_d = nc.dram_tensor("w2c", [CD, 9], F32, kind="ExternalInput")
    w1d_d = nc.dram_tensor("w1d", [4, 3, 128, 128], BF16, kind="ExternalInput")
    w2d_d = nc.dram_tensor("w2d", [4, 5, 128, 128], BF16, kind="ExternalInput")
    bp_d = nc.dram_tensor("bp", [CD, 1], F32, kind="ExternalInput")
    rm_d = nc.dram_tensor("rm", [128, 8], F32, kind="ExternalInput")
    edge_d = nc.dram_tensor("edge", [128, 2], F32, kind="ExternalInput")
    out_d = nc.dram_tensor("outT", [128, 4, NLOC], F32, kind="ExternalOutput")
    vm_d = nc.dram_tensor("vmd", [128, 4, NLOC], F32)      # internal scratch
    o2_d = nc.dram_tensor("o2d", [128, 4, NLOC], BF16)      # internal scratch

    cpool = ctx.enter_context(tc.tile_pool(name="consts", bufs=1))
    wv = [cpool.tile([128, CD], F32, tag=f"wv{m}", name=f"wv{m}") for m in range(4)]
    w1c = [cpool.tile([128, 9], F32, tag=f"w1c{j}", name=f"w1c{j}") for j in range(4)]
    w2c = [cpool.tile([128, 9], F32, tag=f"w2c{j}", name=f"w2c{j}") for j in range(4)]
    bpc = [cpool.tile([128, 1], F32, tag=f"bp{j}", name=f"bp{j}") for j in range(4)]
    edget = cpool.tile([128, 2], F32)
    w1dg = [[cpool.tile([128, 128], BF16, tag=f"w1d{j}{d}", name=f"w1d{j}{d}") for d in range(3)] for j in range(4)]
    w2dg = [[cpool.tile([128, 128], BF16, tag=f"w2d{j}{d}", name=f"w2d{j}{d}") for d in range(5)] for j in range(4)]
    for m in range(4):
        nc.sync.dma_start(wv[m][:], wv_d[m * 128:(m + 1) * 128, :])
        nc.sync.dma_start(w1c[m][:], w1c_d[m * 128:(m + 1) * 128, :])
        nc.sync.dma_start(w2c[m][:], w2c_d[m * 128:(m + 1) * 128, :])
        nc.sync.dma_start(bpc[m][:], bp_d[m * 128:(m + 1) * 128, :])
        for d in range(3):
            nc.sync.dma_start(w1dg[m][d][:], w1d_d[m, d])
        for d in range(5):
            nc.sync.dma_start(w2dg[m][d][:], w2d_d[m, d])
    nc.sync.dma_start(edget[:], edge_d[:])

    gpool = ctx.enter_context(tc.tile_pool(name="gws", bufs=1))
    vpool = tc.alloc_tile_pool(name="vt", bufs=1)
    vbf = [vpool.tile([128, NEXT], BF16, tag=f"vbf{j}", name=f"vbf{j}") for j in range(4)]
    epool = tc.alloc_tile_pool(name="early", bufs=1)
    ident = epool.tile([128, 128], F32, tag="ident", name="ident")
    nc.sync.dma_start(ident[:], ident_d[:])
    wkq = [epool.tile([128, 1024], F32, tag=f"wkq{m}", name=f"wkq{m}") for m in range(4)]
    wph = [epool.tile([DH, CD], F32, tag=f"wp{h}", name=f"wp{h}") for h in range(HEADS)]
    rmt = epool.tile([128, 8], F32, tag="rmt", name="rmt")
    for m in range(4):
        nc.sync.dma_start(wkq[m][:], wkq_d[m * 128:(m + 1) * 128, :])
    for h in range(HEADS):
        nc.sync.dma_start(wph[h][:], wp_d[h])
    nc.sync.dma_start(rmt[:], rm_d[:])

    # ---------- Phase 1: C = X^T X ----------
    pc = tc.alloc_tile_pool(name="pc", bufs=1, space="PSUM")
    xpool = tc.alloc_tile_pool(name="xn", bufs=4)
    c_ps = [pc.tile([128, CD], F32, tag=f"c{i}", name=f"c{i}") for i in range(4)]
    NT = 16  # 16 big tiles of [512 rows, 512]
    for t in range(NT):
        xt_big = xpool.tile([128, 4, CD], F32, tag="xnb", name="xnb")
        nc.sync.dma_start(xt_big[:], xn_d.rearrange("(t p) c -> p t c", p=128)[:, 4 * t:4 * t + 4, :])
        for q in range(4):
            for i in range(4):
                nc.tensor.matmul(c_ps[i][:], xt_big[:, q, 128 * i:128 * (i + 1)], xt_big[:, q, :],
                                 start=(t == 0 and q == 0), stop=(t == NT - 1 and q == 3))
    c_sb = [epool.tile([128, CD], F32, tag=f"csb{i}", name=f"csb{i}") for i in range(4)]
    for i in range(4):
        nc.scalar.activation(c_sb[i][:], c_ps[i][:], AF.Copy)
    xpool.release(); pc.release()
    with tc.tile_pool(name="dram", bufs=1, space="DRAM") as dpool:
        ccin = dpool.tile([CD, CD], F32)
        ccout = dpool.tile([CD, CD], F32)
        for i in range(4):
            nc.sync.dma_start(ccin[128 * i:128 * (i + 1), :], c_sb[i][:])
        nc.gpsimd.collective_compute(
            "AllReduce", OP.add,
            replica_groups=[[0, 1], [2, 3], [4, 5], [6, 7]],
            ins=[ccin.opt()], outs=[ccout.opt()])
        call = [epool.tile([128, CD], F32, tag=f"call{i}", name=f"call{i}") for i in range(4)]
        for i in range(4):
            nc.sync.dma_start(call[i][:], ccout[128 * i:(i + 1) * 128, :])

    # ---------- Phase 2: v^T matmul -> vbf (bf16, ext) + vm (bf16 -> DRAM) ----------
    spool = tc.alloc_tile_pool(name="slab", bufs=4)
    mpool = tc.alloc_tile_pool(name="mslab", bufs=2)
    pv = tc.alloc_tile_pool(name="pv", bufs=3, space="PSUM")
    vmpool = tc.alloc_tile_pool(name="vmt", bufs=4)
    for s in range(17):
        xslab = spool.tile([128, 4, 512], F32, tag="xts", name="xts")
        nc.sync.dma_start(xslab[:], xt_d.rearrange("(m p) n -> p m n", p=128)[:, :, 512 * s:512 * (s + 1)])
        if s < 16:
            mslab = mpool.tile([128, 4, 512], BF16, tag="mks", name="mks")
            nc.sync.dma_start(mslab[:], mk_d.rearrange("(m p) n -> p m n", p=128)[:, :, 512 * s:512 * (s + 1)])
        for j in range(4):
            ps = pv.tile([128, 512], F32, tag="pvt", name="pvt")
            for m in range(4):
                nc.tensor.matmul(ps[:], wv[m][:, 128 * j:128 * (j + 1)], xslab[:, m, :],
                                 start=(m == 0), stop=(m == 3))
            if s < 16:
                nc.scalar.activation(vbf[j][:, 256 + 512 * s: 256 + 512 * (s + 1)], ps[:], AF.Copy)
                vmt = vmpool.tile([128, 512], F32, tag="vmtile", name="vmtile")
                nc.vector.tensor_tensor(vmt[:], ps[:], mslab[:, j, :], OP.mult)
                nc.sync.dma_start(vm_d[:, j, 512 * s:512 * (s + 1)], vmt[:])
            else:
                nc.scalar.activation(vbf[j][:, 0:256], ps[:, 0:256], AF.Copy)
                nc.scalar.activation(vbf[j][:, NEXT - 256:NEXT], ps[:, 256:512], AF.Copy)

    vmpool.release(); pv.release(); mpool.release(); spool.release()

    # ---------- Phase 3: G, norms, softmax, M ----------
    pg = tc.alloc_tile_pool(name="pg", bufs=1, space="PSUM")
    tpool = tc.alloc_tile_pool(name="tmps", bufs=3)
    kqs = epool.tile([128, 8], F32)     # per-head col: rows 0:64 ssq_k, 64:128 ssq_q
    g_sb = [epool.tile([128, 128], F32, tag=f"g{h}", name=f"g{h}") for h in range(HEADS)]
    for h in range(HEADS):
        tsh = [tpool.tile([128, 128], F32, tag=f"tsh{i}", name=f"tsh{i}") for i in range(4)]
        for i in range(4):
            pst = pg.tile([128, 128], F32, tag="pst", name="pst")
            for m in range(4):
                nc.tensor.matmul(pst[:], call[m][:, 128 * i:128 * (i + 1)], wkq[m][:, 128 * h:128 * (h + 1)],
                                 start=(m == 0), stop=(m == 3))
            nc.scalar.activation(tsh[i][:], pst[:], AF.Copy)
        psg = pg.tile([128, 128], F32, tag="psg", name="psg")
        for m in range(4):
            nc.tensor.matmul(psg[:], wkq[m][:, 128 * h:128 * (h + 1)], tsh[m][:],
                             start=(m == 0), stop=(m == 3))
        nc.scalar.activation(g_sb[h][:], psg[:], AF.Copy)
        dtmp = tpool.tile([128, 128], F32, tag="dtmp", name="dtmp")
        nc.vector.tensor_tensor(dtmp[:], g_sb[h][:], ident[:], OP.mult)
        nc.vector.reduce_sum(kqs[:, h:h + 1], dtmp[:], axis=mybir.AxisListType.X)
    # inv-norm with eps and one Newton step; fold rescale into k-side
    nrm = epool.tile([128, 8], F32)
    inv = epool.tile([128, 8], F32)
    nc.scalar.activation(nrm[:], kqs[:], AF.Sqrt)
    nc.vector.tensor_scalar_max(nrm[:], nrm[:], EPS)
    nc.vector.reciprocal(inv[:], nrm[:])
    t_a = epool.tile([128, 8], F32)
    nc.vector.tensor_tensor(t_a[:], inv[:], inv[:], OP.mult)
    nc.vector.tensor_tensor(t_a[:], t_a[:], kqs[:], OP.mult)
    nc.vector.tensor_scalar(t_a[:], t_a[:], -0.5, 1.5, OP.mult, OP.add)
    nc.vector.tensor_tensor(inv[:], inv[:], t_a[:], OP.mult)
    nc.vector.tensor_tensor(inv[:], inv[:], rmt[:], OP.mult)  # rescale on k rows, 1.0 on q rows
    # per head: ZT = G[64:,0:64]*qs -> PE transpose -> Z*ks -> softmax -> A; M via A,Wp
    m_sb = [gpool.tile([128, CD], F32, tag=f"msb{j}", name=f"msb{j}") for j in range(4)]
    for h in range(HEADS):
        zt = tpool.tile([128, 64], F32, tag="zt", name="zt")
        nc.vector.tensor_scalar(zt[64:128, :], g_sb[h][64:128, 0:64], inv[64:128, h:h + 1], None, OP.mult)
        zps = pg.tile([128, 64], F32, tag="zps", name="zps")
        nc.tensor.transpose(zps[0:64, :], zt[64:128, :], ident[64:128, 64:128])
        z = tpool.tile([64, 64], F32, tag="z", name="z")
        nc.vector.tensor_scalar(z[:], zps[0:64, 0:64], inv[0:64, h:h + 1], None, OP.mult)
        rmx = tpool.tile([64, 1], F32, tag="rmx", name="rmx")
        nc.vector.reduce_max(rmx[:], z[:], axis=mybir.AxisListType.X)
        nc.vector.tensor_scalar(rmx[:], rmx[:], -1.0, None, OP.mult)
        ez = tpool.tile([64, 64], F32, tag="ez", name="ez")
        nc.scalar.activation(ez[:], z[:], AF.Exp, bias=rmx[:])
        sm = tpool.tile([64, 1], F32, tag="sm", name="sm")
        nc.vector.reduce_sum(sm[:], ez[:], axis=mybir.AxisListType.X)
        rs = tpool.tile([64, 1], F32, tag="rs", name="rs")
        nc.vector.reciprocal(rs[:], sm[:])
        a_t = tpool.tile([64, 64], F32, tag="at", name="at")
        nc.vector.tensor_scalar(a_t[:], ez[:], rs[:], None, OP.mult)
        # M_h^T[e, cout] = sum_d A[d, e] * Wp[(h,d), cout]
        mps = pg.tile([64, CD], F32, tag="mps", name="mps")
        nc.tensor.matmul(mps[:], a_t[:], wph[h][:], start=True, stop=True)
        j = h // 2
        if h % 2 == 0:
            nc.scalar.activation(m_sb[j][0:64, :], mps[:], AF.Copy)
        else:
            mstg = tpool.tile([64, CD], F32, tag="mstg", name="mstg")
            nc.scalar.activation(mstg[:], mps[:], AF.Copy)
            nc.sync.dma_start(m_sb[j][64:128, :], mstg[:])  # partition shift via DMA

    tpool.release(); pg.release(); epool.release()

    # ---------- Phase 4: conv1 (PE dy-taps + DVE x-taps), gelu ----------
    c1pool = tc.alloc_tile_pool(name="c1", bufs=2)
    pcv = tc.alloc_tile_pool(name="pcv", bufs=3, space="PSUM")
    o2pool = tc.alloc_tile_pool(name="o2", bufs=2)

    for j in range(4):
        out1j = c1pool.tile([128, 8448], BF16, tag="o1t", name="o1t")
        gtj = c1pool.tile([128, 8448], BF16, tag="gtt", name="gtt")
        # PE: dy taps (dx=0). out1 cols [512t, 512t+512), t=16 -> 256 wide
        for t in range(17):
            wdt = 512 if t < 16 else 256
            ps = pcv.tile([128, 512], F32, tag="pc1", name="pc1")
            for di, dy in enumerate((-1, 0, 1)):
                # natural ext col = out1col + 128*(1+dy); remap: core cols at +256 base... out1 row io maps to ext row io+1+dy
                base = 512 * t + 128 * (1 + dy)
                # vbf layout: [halo-lo 0:256 | core 256:8448+... ] natural: ext row ie at col 128*ie
                nc.tensor.matmul(ps[:, 0:wdt], w1dg[j][di][:], vbf[j][:, base:base + wdt],
                                 start=(di == 0), stop=(di == 2))
            nc.scalar.activation(out1j[:, 512 * t:512 * t + wdt], ps[:, 0:wdt], AF.Copy)
        o1v = out1j.rearrange("p (y x) -> p y x", x=128)
        vv = vbf[j].rearrange("p (y x) -> p y x", x=128)
        for dy in (-1, 0, 1):
            for dx in (-1, 1):
                k = (dy + 1) * 3 + (dx + 1)
                if dx == -1:
                    nc.vector.scalar_tensor_tensor(
                        o1v[:, :, 1:128], vv[:, 1 + dy:67 + dy, 0:127], w1c[j][:, k:k + 1],
                        o1v[:, :, 1:128], OP.mult, OP.add)
                else:
                    nc.vector.scalar_tensor_tensor(
                        o1v[:, :, 0:127], vv[:, 1 + dy:67 + dy, 1:128], w1c[j][:, k:k + 1],
                        o1v[:, :, 0:127], OP.mult, OP.add)
        nc.vector.tensor_scalar(o1v[:, 0:1, :], o1v[:, 0:1, :], edget[:, 0:1], None, OP.mult)
        nc.vector.tensor_scalar(o1v[:, 65:66, :], o1v[:, 65:66, :], edget[:, 1:2], None, OP.mult)
        nc.scalar.activation(gtj[:], out1j[:], AF.Gelu_apprx_tanh)

        # conv2 for this chunk
        o2t = o2pool.tile([128, NLOC], BF16, tag="o2t", name="o2t")
        for t in range(16):
            ps = pcv.tile([128, 512], F32, tag="pc2", name="pc2")
            for di, dy in zip((0, 2, 4), (-1, 0, 1)):
                base = 512 * t + 128 * (1 + dy)
                nc.tensor.matmul(ps[:], w2dg[j][di][:], gtj[:, base:base + 512],
                                 start=(di == 0), stop=False, skip_group_check=True)
            psv = ps.rearrange("p (y x) -> p y x", x=128)
            gsv = gtj.rearrange("p (y x) -> p y x", x=128)
            nc.tensor.matmul(psv[:, :, 1:128], w2dg[j][1][:], gsv[:, 4 * t + 1:4 * t + 5, 0:127],
                             start=False, stop=False, skip_group_check=True)
            nc.tensor.matmul(psv[:, :, 0:127], w2dg[j][3][:], gsv[:, 4 * t + 1:4 * t + 5, 1:128],
                             start=False, stop=True, skip_group_check=True)
            nc.scalar.activation(o2t[:, 512 * t:512 * (t + 1)], ps[:], AF.Copy)
        o2v = o2t.rearrange("p (y x) -> p y x", x=128)
        gv = gtj.rearrange("p (y x) -> p y x", x=128)
        for dy in (-1, 1):
            for dx in (-1, 1):
                k = (dy + 1) * 3 + (dx + 1)
                if dx == -1:
                    nc.vector.scalar_tensor_tensor(
                        o2v[:, :, 1:128], gv[:, 1 + dy:65 + dy, 0:127], w2c[j][:, k:k + 1],
                        o2v[:, :, 1:128], OP.mult, OP.add)
                else:
                    nc.vector.scalar_tensor_tensor(
                        o2v[:, :, 0:127], gv[:, 1 + dy:65 + dy, 1:128], w2c[j][:, k:k + 1],
                        o2v[:, :, 0:127], OP.mult, OP.add)
        nc.sync.dma_start(o2_d[:, j, :], o2t[:])

    o2pool.release(); pcv.release(); c1pool.release(); vpool.release()

    # ---------- Phase 6: attention out + final add ----------
    apool = ctx.enter_context(tc.tile_pool(name="att", bufs=4))
    po = ctx.enter_context(tc.tile_pool(name="po", bufs=6, space="PSUM"))
    for k in range(16):
        vms = apool.tile([128, 4, 512], F32, tag="vms", name="vms")
        nc.sync.dma_start(vms[:], vm_d[:, :, 512 * k:512 * (k + 1)])
        o2s = apool.tile([128, 4, 512], BF16, tag="o2s", name="o2s")
        nc.sync.dma_start(o2s[:], o2_d[:, :, 512 * k:512 * (k + 1)])
        outs = apool.tile([128, 4, 512], F32, tag="outs", name="outs")
        for i in range(4):
            ps = po.tile([128, 512], F32, tag="pso", name="pso")
            for j in range(4):
                nc.tensor.matmul(ps[:], m_sb[j][:, 128 * i:128 * (i + 1)], vms[:, j, :],
                                 start=(j == 0), stop=(j == 3))
            nc.vector.scalar_tensor_tensor(outs[:, i, :], o2s[:, i, :], bpc[i][:],
                                           ps[:], OP.add, OP.add)
        nc.sync.dma_start(out_d[:, :, 512 * k:512 * (k + 1)], outs[:])

    ctx.close()


def _build():
    if "nc" in _cache:
        return _cache["nc"]
    nc = bacc.Bacc("TRN2", target_bir_lowering=False, debug=False, num_devices=NCORES)
    with tile.TileContext(nc) as tc:
        _emit(nc, tc)
    nc.compile()
    _cache["nc"] = nc
    return nc


def _prep_core(core, x_in, mask, Wq, Wk, Wv, rescale, Wp, bp, conv1_w, conv2_w):
    b, half = core // 2, core % 2
    y0 = half * 64
    xi = x_in[b]
    xn = np.ascontiguousarray(xi[y0:y0 + 64].reshape(NLOC, CD))
    # xT ext: [core 8192 | halo-lo 256 | halo-hi 256]
    xt = np.zeros((CD, NEXT), np.float32)
    xt[:, 0:NLOC] = xn.T
    if y0 - 2 >= 0:
        xt[:, NLOC:NLOC + 256] = xi[y0 - 2:y0].reshape(256, CD).T
    if y0 + 66 <= H:
        xt[:, NLOC + 256:] = xi[y0 + 64:y0 + 66].reshape(256, CD).T
    mk = np.ascontiguousarray(mask[b][y0:y0 + 64].reshape(NLOC, CD).T).astype(ml_dtypes.bfloat16)
    wkq = np.zeros((CD, 1024), np.float32)
    for h in range(HEADS):
        wkq[:, 128 * h:128 * h + 64] = Wk[:, DH * h:DH * (h + 1)]
        wkq[:, 128 * h + 64:128 * (h + 1)] = Wq[:, DH * h:DH * (h + 1)]
    w1c = conv1_w.reshape(CD, 9).astype(np.float32)
    w2c = conv2_w.reshape(CD, 9).astype(np.float32)
    w1d = np.zeros((4, 3, 128, 128), np.float32)
    w2d = np.zeros((4, 5, 128, 128), np.float32)
    for j in range(4):
        for di, dy in enumerate((-1, 0, 1)):
            k = (dy + 1) * 3 + 1  # dx = 0 tap
            np.fill_diagonal(w1d[j, di], w1c[128 * j:128 * (j + 1), k])
        for di, k in enumerate((1, 3, 4, 5, 7)):
            np.fill_diagonal(w2d[j, di], w2c[128 * j:128 * (j + 1), k])
    rm = np.ones((128, 8), np.float32)
    rm[0:64, :] = rescale.reshape(1, 8)
    edge = np.ones((128, 2), np.float32)
    if y0 - 1 < 0:
        edge[:, 0] = 0.0
    if y0 + 64 >= H:
        edge[:, 1] = 0.0
    return {
        "ident": np.eye(128, dtype=np.float32),
        "xn": xn, "xt": xt, "mk": mk, "wkq": wkq,
        "wv": Wv.astype(np.float32),
        "wp": Wp.reshape(HEADS, DH, CD).astype(np.float32),
        "w1c": w1c, "w2c": w2c,
        "w1d": w1d.astype(ml_dtypes.bfloat16), "w2d": w2d.astype(ml_dtypes.bfloat16),
        "bp": bp.reshape(CD, 1).astype(np.float32),
        "rm": rm, "edge": edge,
    }


def kernel(**inputs):
    inputs = {k: np.asarray(v) for k, v in inputs.items()}
    nc = _build()
    in_maps = [_prep_core(c, **inputs) for c in range(NCORES)]
    trace = bool(int(os.environ.get("BGMSA_TRACE", "0")))
    try:
        res = run_bass_kernel_spmd(nc, in_maps, list(range(NCORES)), trace=trace)
    except Exception:
        if not trace:
            raise
        res = run_bass_kernel_spmd(nc, in_maps, list(range(NCORES)), trace=False)
    _cache["last_exec_ns"] = res.exec_time_ns
    out = np.zeros((B, H, W, CD), np.float32)
    for c in range(NCORES):
        b, half = c // 2, c % 2
        arr = res.results[c]["outT"]  # [128, 4, 8192]
        outT = np.transpose(arr, (1, 0, 2)).reshape(CD, NLOC)
        out[b, half * 64:half * 64 + 64] = outT.T.reshape(64, W, CD)
    return out

